# revision 80
# baseline (speedup 1.0000x reference)
"""Trainium2 Bass kernel for an MoE routing module.

Strategy: data-parallel over the batch — each of the 8 NeuronCores runs the
full pipeline (gating -> top-2 -> expert MLPs) for its 8 samples. All
data-dependent expert selection happens on device via gathers driven by the
top-2 result; there are no collectives and no registers.

Host-side prep:
  - gating embedding is pre-multiplied through the gate's first layer:
    embG = (emb @ gate_w1) / S (f64 accumulate, f32 store), so the device
    gathers 256-wide rows instead of 1024-wide ones and the gate L1 matmul
    disappears. Top-2 selection needs exact-ish f32 logits (margins are
    ~1e-5), so embG stays f32 and the tiny L2 matmul runs in true f32.
  - per-core COMPACT expert-embedding table: a core touches at most
    BL*S = 4096 distinct vocab ids, so the host dedupes them and ships
    [E*4096, D] in e4m3 (*FP8_SCALE). (e, slot) indices then fit int16,
    which lets ONE gpsimd dma_gather fetch all 512 token rows per (b,k)
    (vs 4 indirect DMAs whose SWDGE descriptor prep made the Pool engine
    the pacing engine).
  - expert weights are packed per-expert into ONE fp8 "mega table"
    [E*128, WXCOLS]: W1 as e4m3*FP8_SCALE (t-major d-tiles), then the raw
    bytes of a bf16 side table (W2 hi+lo pair so W2 reconstructs to ~f32,
    b1 pre-scaled into the unscaled-z domain, b2). A [128,1] index tile
    (value e*128+p) gathers everything for an expert in a single indirect
    DMA with 128 fat descriptors.
Expert math: tok/W1 fp8 with DoubleRow matmuls (K=256 per instr, fp32 PSUM);
tok is transposed d-major via PE identity matmuls with the psum->fp8 copies
split across DVE and the scalar engine (RELU_ENG/COPY_ENG balance the
per-[128,512]-tile relu+accum and copy work across both). The FP8_SCALE^-2
and 1/S factors fold into the pooled-vector scale together with the routing
weight; b2 and the routing weight fold into one scalar_tensor_tensor op.
The gating chain is kept short (it gates the pipelined expert loop): h^T is
produced directly in partition layout by N=1 partition-sum matmuls, gb2 is
a rank-1 matmul into the logit psum, top-2 reads the logit-transpose psum
directly, and the per-(b,k) scalars are broadcast to all partitions with
one-hot-row lhsT matmuls instead of a DRAM bounce.

HW gotchas (verified on device): indirect DMA consumes exactly ONE index
per destination partition; walrus rejects DVE tensor_tensor with two PSUM
operands, f32r matmul inputs that aren't produced as f32r, and scale+bias+
accum_out all on one activation (runtime failure).
"""

import os
import sys

for _p in ("/opt/trn_rl_repo", "/root/.axon_site/_ro/trn_rl_repo"):
    if os.path.isdir(_p) and _p not in sys.path:
        sys.path.insert(0, _p)

import numpy as np

import concourse.bacc as bacc
import concourse.tile as tile
import concourse.mybir as mybir
from concourse.bass import IndirectOffsetOnAxis
from concourse.bass_utils import run_bass_kernel_spmd
from concourse.masks import make_identity

F32 = mybir.dt.float32
F32R = mybir.dt.float32r
BF16 = mybir.dt.bfloat16
F8 = mybir.dt.float8e4
I32 = mybir.dt.int32
I16 = mybir.dt.int16
U32 = mybir.dt.uint32

V, D, H, E, C, TOPK = 16000, 1024, 1024, 8, 16, 2
B, S = 64, 512
GATE_H = 256
NCORES = 8
BL = B // NCORES          # samples per core
DT = D // 128             # 8 d-tiles
HT = H // 128             # 8 h-tiles
ST = S // 128             # 4 s-tiles
MT = GATE_H // 128        # 2 gate-hidden tiles
NGRP = 4                  # sample groups per core (pipelining)
GBL = BL // NGRP          # samples per group

USE_DOUBLE_ROW = True     # fp8 DoubleRow (K=256/matmul); False = normal fp8
RELU_ENG = "AAAAAADD"     # per h-tile: A=scalar, D=DVE relu+accum engine
COPY_ENG = "DDDDDDDA"     # per d-tile: A=scalar, D=DVE, P=gpsimd tokT copy
W2F_ENG = "D"             # engine for the W2 hi+lo add (D=DVE, P=gpsimd)
U = 4096                  # per-core compact vocab (8 samples x 512 tokens)

# fp8 scaling: tok and W1 stored as e4m3 * FP8_SCALE; z_psum carries
# FP8_SCALE^2, divided out in the pooled-vector scale.
FP8_SCALE = 64.0
FP8_UNSCALE = 1.0 / (FP8_SCALE * FP8_SCALE)

# bf16 side table layout (within the fp8 mega table, bytes after W1)
W2COL = 0                 # W2 hi [HT*C]
W2LO = W2COL + HT * C     # 128   W2 lo [HT*C]
B1COL = W2LO + HT * C     # 256   b1 * FP8_SCALE^2 [HT]
B1NEG = B1COL + HT        # 264   -b1 * FP8_SCALE^2 [HT]
B2COL = B1NEG + HT        # 272   b2 (partitions 0..C-1) [1]
WSMCOLS = 288             # padded bf16 row length
W1OFF = DT * H            # 8192 fp8 bytes of W1
WXCOLS = W1OFF + 2 * WSMCOLS  # 8768 fp8 row length of the mega table

_compiled = {}
last_results = None       # BassKernelResults of the most recent run (for test.py)


def build_program(reps=1):
    """reps>1 repeats the whole compute body (benchmarking aid)."""
    nc = bacc.Bacc("TRN2", target_bir_lowering=False, debug=False, num_devices=NCORES)
    act = mybir.ActivationFunctionType

    xw_t = nc.dram_tensor("xw16", [128, BL, S // 16], I16, kind="ExternalInput")
    xws_t = nc.dram_tensor("xws16", [128, BL, S // 16], I16, kind="ExternalInput")
    embg_t = nc.dram_tensor("embg", [V, GATE_H], F32, kind="ExternalInput")
    ctab_t = nc.dram_tensor("ctab", [E * U, D], F8, kind="ExternalInput")
    wx_t = nc.dram_tensor("wx", [E * 128, WXCOLS], F8, kind="ExternalInput")
    gb1_t = nc.dram_tensor("gb1", [128, MT], F32, kind="ExternalInput")
    gw2_t = nc.dram_tensor("gw2", [GATE_H, E], F32, kind="ExternalInput")
    gb2_t = nc.dram_tensor("gb2r", [1, E], F32, kind="ExternalInput")
    eyebl_t = nc.dram_tensor("eyebl", [GBL, GBL * 128], F32, kind="ExternalInput")
    out_t = nc.dram_tensor("out", [BL, C], F32, kind="ExternalOutput")

    with tile.TileContext(nc) as tc:
        with (
            tc.tile_pool(name="const", bufs=1) as cpool,
            tc.tile_pool(name="dram", bufs=1, space="DRAM") as dpool,
        ):
            # ---- constants ----
            id_f8 = cpool.tile([128, 128], F8)
            make_identity(nc, id_f8[:, :])
            id_f = cpool.tile([128, 128], F32)
            make_identity(nc, id_f[:, :])
            ones_k = cpool.tile([128, 1], F32)      # lhsT for partition-sum MMs
            nc.vector.memset(ones_k[:, :], 1.0)

            ones_m = cpool.tile([1, 128], F32)      # lhsT for K=1 broadcast MMs
            nc.vector.memset(ones_m[:, :], 1.0)
            iota_p = cpool.tile([128, 1], I32)      # value = partition index
            nc.gpsimd.iota(iota_p[:, :], pattern=[[0, 1]], base=0, channel_multiplier=1)
            zero_c = cpool.tile([128, 1], F32)
            nc.vector.memset(zero_c[:, :], 0.0)

            # int16 wrapped indices for dma_gather (pre-wrapped on host):
            # xw = raw vocab ids (gating), xws = compact slot ids (experts)
            xw = cpool.tile([128, BL, S // 16], I16)
            nc.sync.dma_start(out=xw[:, :, :], in_=xw_t[:, :, :])
            xws = cpool.tile([128, BL, S // 16], I16)
            nc.sync.dma_start(out=xws[:, :, :], in_=xws_t[:, :, :])

            gb1_sb = cpool.tile([128, MT], F32)
            nc.sync.dma_start(out=gb1_sb[:, :], in_=gb1_t[:, :])
            gb2_sb = cpool.tile([1, E], F32)
            nc.sync.dma_start(out=gb2_sb[:, :], in_=gb2_t[:, :])
            eyebl_sb = cpool.tile([GBL, GBL * 128], F32)
            nc.sync.dma_start(out=eyebl_sb[:, :], in_=eyebl_t[:, :])
            gw2_sb = cpool.tile([128, MT, E], F32)
            nc.sync.dma_start(
                out=gw2_sb[:, :, :], in_=gw2_t[:, :].rearrange("(m p) e -> p m e", p=128)
            )

            consts = dict(
                id_f8=id_f8, id_f=id_f, ones_k=ones_k,
                ones_m=ones_m, iota_p=iota_p, zero_c=zero_c, xw=xw, xws=xws,
                gb1_sb=gb1_sb, gb2_sb=gb2_sb, gw2_sb=gw2_sb, eyebl_sb=eyebl_sb,
            )
            tensors = dict(
                embg_t=embg_t, ctab_t=ctab_t, wx_t=wx_t, out_t=out_t,
            )
            # chain tile serializes reps so the benchmark differential is honest
            chain = None
            if reps > 1:
                chain = cpool.tile([1, 1], F32)
                nc.vector.memset(chain[:, :], 0.0)
            for rep in range(reps):
                _body_once(nc, tc, act, rep, dpool, consts, tensors, chain)

    nc.compile()
    return nc


def _body_once(nc, tc, act, rep, dpool, cn, tn, chain=None):
    sfx = f"_r{rep}"
    id_f8, id_f = cn["id_f8"], cn["id_f"]
    ones_k, ones_m, iota_p, zero_c = cn["ones_k"], cn["ones_m"], cn["iota_p"], cn["zero_c"]
    xw, xws = cn["xw"], cn["xws"]
    gb1_sb, gb2_sb, gw2_sb = cn["gb1_sb"], cn["gb2_sb"], cn["gw2_sb"]
    eyebl_sb = cn["eyebl_sb"]
    embg_t, ctab_t, wx_t, out_t = tn["embg_t"], tn["ctab_t"], tn["wx_t"], tn["out_t"]

    with (
        tc.tile_pool(name=f"persist{sfx}", bufs=1) as ppool,
        tc.tile_pool(name=f"bc{sfx}", bufs=2) as bcpool,
        # gating pools
        tc.tile_pool(name=f"gat{sfx}", bufs=4) as gpool,
        tc.tile_pool(name=f"gsb{sfx}", bufs=2) as gspool,
        tc.tile_pool(name=f"gps{sfx}", bufs=1, space="PSUM") as gps,
        tc.tile_pool(name=f"gpss{sfx}", bufs=1, space="PSUM") as gps_s,
        # expert pools
        tc.tile_pool(name=f"exi{sfx}", bufs=3) as xipool,
        tc.tile_pool(name=f"etok{sfx}", bufs=3) as tokpool,
        tc.tile_pool(name=f"ew{sfx}", bufs=3) as wpool,
        tc.tile_pool(name=f"ett{sfx}", bufs=3) as ttpool,
        tc.tile_pool(name=f"esm{sfx}", bufs=3) as smpool,
        tc.tile_pool(name=f"ejunk{sfx}", bufs=2) as junkpool,
        tc.tile_pool(name=f"epst{sfx}", bufs=2, space="PSUM") as eps_t,
        tc.tile_pool(name=f"epsz{sfx}", bufs=3, space="PSUM") as eps_z,
        tc.tile_pool(name=f"epso{sfx}", bufs=1, space="PSUM") as eps_o,
    ):
        out_acc = ppool.tile([C, BL], F32)
        nc.vector.memset(out_acc[:, :], 0.0)

        # deferred W2 tail of the previous (b,k): emitting it after the next
        # (b,k)'s GEMMs keeps the in-order PE queue from stalling on the
        # relu->psc dependency
        pending = []

        def flush_tail():
            if not pending:
                return
            st = pending.pop()
            psc = smpool.tile([128, HT], F32, tag="psc")
            nc.vector.scalar_tensor_tensor(
                out=psc[:, :],
                in0=st["pacc"][:, :],
                scalar=FP8_UNSCALE / S,
                in1=st["BCf"][:, st["cRW"] : st["cRW"] + 1].to_broadcast([128, HT]),
                op0=mybir.AluOpType.mult,
                op1=mybir.AluOpType.mult,
            )
            eo_ps = eps_o.tile([C, 1], F32, tag="eo")
            for j2 in range(HT):
                nc.tensor.matmul(
                    out=eo_ps[:, :],
                    lhsT=st["w2f"][:, j2 * C : (j2 + 1) * C],
                    rhs=psc[:, j2 : j2 + 1],
                    start=(j2 == 0),
                    stop=(j2 == HT - 1),
                )
            # out_acc[:, b] += rw*(p@W2) + rw*b2: psc already carries rw, so
            # add rw*b2 via stt: (b2 mult rw) add eo
            eo2 = smpool.tile([C, 1], F32, tag="eo2")
            nc.vector.scalar_tensor_tensor(
                out=eo2[:, :],
                in0=st["b2f"][:, :],
                scalar=st["BCf"][0:C, st["cRW"] : st["cRW"] + 1],
                in1=eo_ps[:, :],
                op0=mybir.AluOpType.mult,
                op1=mybir.AluOpType.add,
            )
            b = st["b"]
            nc.vector.tensor_add(
                out_acc[:, b : b + 1], out_acc[:, b : b + 1], eo2[:, :]
            )

        for g in range(NGRP):
            b0 = g * GBL
            # ============ gating for samples [b0, b0+GBL) (f32) ============
            # hT[p, m] = relu(pooled @ gw1 + gb1)[m*128+p], computed directly
            # in partition layout: 8 tiny N=1 matmuls sum gtok g-slices over
            # tokens (f32r; m13 truncation is ~1e-8 on the logits, margins
            # are ~1e-5).
            hTs = gspool.tile([128, MT, GBL], F32, tag="hTs")
            for bl in range(GBL):
                b = b0 + bl
                # embG rows for this sample's tokens: [128, ST, 256]
                gtok = gpool.tile([128, ST, GATE_H], F32, tag="gtok")
                nc.gpsimd.dma_gather(
                    out_ap=gtok[:, :, :],
                    in_ap=embg_t[:, :],
                    idxs_ap=xw[:, b, :],
                    num_idxs=S,
                    num_idxs_reg=S,
                    elem_size=GATE_H,
                    transpose=False,
                )
                hp = gps.tile([128, MT], F32, tag="pp")
                for m in range(MT):
                    for t in range(ST):
                        nc.tensor.matmul(
                            out=hp[:, m : m + 1],
                            lhsT=gtok[:, t, m * 128 : (m + 1) * 128],
                            rhs=ones_k[:, :],
                            start=(t == 0),
                            stop=(t == ST - 1),
                        )
                # h = relu(hp + gb1)  (1/S is folded into embG on host)
                aT = gspool.tile([128, MT], F32, tag="aT")
                nc.vector.tensor_add(aT[:, :], hp[:, :], gb1_sb[:, :])
                nc.vector.tensor_scalar_max(hTs[:, :, bl], aT[:, :], 0.0)

            # gate layer 2 + gb2 (rank-1 matmul) -> logits [e, b], then
            # transpose to [b, e]; all on PE so the chain stays short
            l_ps = gps_s.tile([E, GBL], F32, tag="gmisc")
            for m in range(MT):
                nc.tensor.matmul(
                    out=l_ps[:, :],
                    lhsT=gw2_sb[:, m, :],
                    rhs=hTs[:, m, :],
                    start=(m == 0),
                    stop=False,
                )
            nc.tensor.matmul(
                out=l_ps[:, :],
                lhsT=gb2_sb[0:1, :],
                rhs=ones_m[0:1, 0:GBL],
                start=False,
                stop=True,
            )
            l_sb = gspool.tile([E, GBL], F32, tag="l_sb")
            nc.vector.tensor_copy(l_sb[:, :], l_ps[:, :])
            lt_ps = gps_s.tile([GBL, E], F32, tag="gmisc")
            nc.tensor.matmul(
                out=lt_ps[:, :], lhsT=l_sb[:, :], rhs=id_f[0:E, 0:E],
                start=True, stop=True,
            )

            # top-2 of logits == top-2 of softmax (monotone); DVE reads the
            # psum tile directly
            mx = gspool.tile([GBL, 8], F32, tag="mx")
            mi = gspool.tile([GBL, 8], U32, tag="mi")
            nc.vector.max_with_indices(mx[:, :], mi[:, :], lt_ps[:, :])

            # renormalized top-2 softmax weights:
            # rw1 = 1/(1+exp(l2-l1)), rw2 = exp(l2-l1)/(1+exp(l2-l1))
            dlt = gspool.tile([GBL, 1], F32, tag="dlt")
            nc.vector.tensor_sub(dlt[:, :], mx[:, 1:2], mx[:, 0:1])
            q = gspool.tile([GBL, 1], F32, tag="q")
            nc.scalar.activation(out=q[:, :], in_=dlt[:, :], func=act.Exp)
            sden = gspool.tile([GBL, 1], F32, tag="sden")
            nc.vector.tensor_scalar_add(sden[:, :], q[:, :], 1.0)
            rw1 = gspool.tile([GBL, 1], F32, tag="rw1")
            nc.vector.reciprocal(rw1[:, :], sden[:, :])
            rw2 = gspool.tile([GBL, 1], F32, tag="rw2")
            nc.vector.tensor_mul(rw2[:, :], q[:, :], rw1[:, :])

            # pack per-(b,k) scalars: cols bl*8 + {0,1}=e*U, {2,3}=e*128,
            # {6,7}=rw ({4,5} unused)
            ei_f = gspool.tile([GBL, TOPK], F32, tag="ei_f")
            nc.vector.tensor_copy(ei_f[:, :], mi[:, 0:TOPK])
            vals = gspool.tile([GBL, 8], F32, tag="vals")
            nc.vector.tensor_scalar_mul(vals[:, 0:2], ei_f[:, :], float(U))
            nc.vector.tensor_scalar_mul(vals[:, 2:4], ei_f[:, :], 128.0)
            nc.vector.tensor_scalar_mul(vals[:, 4:6], ei_f[:, :], 0.0)
            nc.vector.tensor_copy(vals[:, 6:7], rw1[:, :])
            nc.vector.tensor_copy(vals[:, 7:8], rw2[:, :])

            # broadcast vals[bl, :] to all partitions of cols bl*8..bl*8+8
            # via one-hot-row lhsT matmuls (no DRAM bounce)
            if chain is not None:
                # unused col 4: forces rep r to wait on rep r-1's result
                nc.vector.tensor_copy(vals[0:1, 4:5], chain[0:1, 0:1])
            bc_ps = gps_s.tile([128, GBL * 8], F32, tag="gmisc")
            for bl in range(GBL):
                nc.tensor.matmul(
                    out=bc_ps[:, bl * 8 : (bl + 1) * 8],
                    lhsT=eyebl_sb[:, bl * 128 : (bl + 1) * 128],
                    rhs=vals[:, :],
                    start=True,
                    stop=True,
                )
            BCf = bcpool.tile([128, GBL * 8], F32, tag="bcf")
            BCi = bcpool.tile([128, GBL * 8], I32, tag="bci")
            BCi16 = bcpool.tile([128, GBL * 8], I16, tag="bci16")
            nc.vector.tensor_copy(BCf[:, :], bc_ps[:, :])
            nc.vector.tensor_copy(BCi[:, :], bc_ps[:, :])    # cast f32->i32
            nc.vector.tensor_copy(BCi16[:, :], bc_ps[:, :])  # cast f32->i16

            # ============ experts for this group (fp8) ============
            for bl in range(GBL):
                b = b0 + bl
                for k in range(TOPK):
                    cEV = bl * 8 + k
                    cE128 = bl * 8 + 2 + k
                    cRW = bl * 8 + 6 + k

                    # compact-table indices: slot + e*U (fits int16: <= 32763)
                    tok_idx = xipool.tile([128, S // 16], I16, tag="tok_idx")
                    nc.vector.tensor_add(
                        tok_idx[:, :],
                        xws[:, b, :],
                        BCi16[:, cEV : cEV + 1].to_broadcast([128, S // 16]),
                    )
                    w_idx = xipool.tile([128, 1], I32, tag="w_idx")
                    nc.vector.tensor_add(
                        w_idx[:, :], iota_p[:, :], BCi[:, cE128 : cE128 + 1]
                    )

                    tok = tokpool.tile([128, ST, D], F8, tag="tok")
                    nc.gpsimd.dma_gather(
                        out_ap=tok[:, :, :],
                        in_ap=ctab_t[:, :],
                        idxs_ap=tok_idx[:, :],
                        num_idxs=S,
                        num_idxs_reg=S,
                        elem_size=D,
                        transpose=False,
                    )
                    # one gather for W1 (fp8) + bf16 side table (as raw bytes)
                    wg = wpool.tile([128, WXCOLS], F8, tag="wg")
                    nc.gpsimd.indirect_dma_start(
                        out=wg[:, :],
                        out_offset=None,
                        in_=wx_t[:, :],
                        in_offset=IndirectOffsetOnAxis(ap=w_idx[:, :], axis=0),
                    )
                    wsm = wg[:, W1OFF:].bitcast(BF16)      # [128, WSMCOLS] bf16
                    b1un = smpool.tile([128, 2 * HT], F32, tag="b1un")
                    nc.vector.tensor_copy(b1un[:, :], wsm[:, B1COL : B1COL + 2 * HT])
                    b1u = b1un[:, 0:HT]
                    b1n = b1un[:, HT : 2 * HT]
                    b2f = smpool.tile([C, 1], F32, tag="b2f")
                    nc.vector.tensor_copy(b2f[:, :], wsm[0:C, B2COL : B2COL + 1])
                    w2f = smpool.tile([128, HT * C], F32, tag="w2f")
                    w2f_eng = nc.gpsimd if W2F_ENG == "P" else nc.vector
                    w2f_eng.tensor_add(
                        w2f[:, :], wsm[:, W2COL : W2COL + HT * C],
                        wsm[:, W2LO : W2LO + HT * C],
                    )

                    # transpose tok -> tokT[d, s] via matmul against identity
                    tokT = ttpool.tile([128, DT, S], F8, tag="tokT")
                    for j in range(DT):
                        tp = eps_t.tile([128, S], F32, tag="tp")
                        for t in range(ST):
                            nc.tensor.matmul(
                                out=tp[:, t * 128 : (t + 1) * 128],
                                lhsT=tok[:, t, j * 128 : (j + 1) * 128],
                                rhs=id_f8[:, :],
                                start=True,
                                stop=True,
                            )
                        # split psum->fp8 copies across DVE/scalar/gpsimd
                        if COPY_ENG[j] == "D":
                            nc.vector.tensor_copy(tokT[:, j, :], tp[:, :])
                        elif COPY_ENG[j] == "A":
                            nc.scalar.copy(tokT[:, j, :], tp[:, :])
                        else:
                            nc.gpsimd.tensor_copy(tokT[:, j, :], tp[:, :])

                    # z[h_tile] = relu(tokT.T @ W1 + b1*SC^2); accumulate sum
                    # over s. fp8 DoubleRow contracts 2 d-tiles per matmul.
                    w1v = wg[:, 0:W1OFF].rearrange("p (t h) -> p t h", t=DT)
                    pacc = smpool.tile([128, HT], F32, tag="pacc")
                    kstep = 2 if USE_DOUBLE_ROW else 1
                    pmode = mybir.MatmulPerfMode.DoubleRow if USE_DOUBLE_ROW else None
                    for j2 in range(HT):
                        z_ps = eps_z.tile([128, S], F32, tag="z")
                        for t in range(0, DT, kstep):
                            if USE_DOUBLE_ROW:
                                lhsT = w1v[:, t : t + 2, j2 * 128 : (j2 + 1) * 128]
                                rhs = tokT[:, t : t + 2, :]
                            else:
                                lhsT = w1v[:, t, j2 * 128 : (j2 + 1) * 128]
                                rhs = tokT[:, t, :]
                            nc.tensor.matmul(
                                out=z_ps[:, :],
                                lhsT=lhsT,
                                rhs=rhs,
                                start=(t == 0),
                                stop=(t == DT - kstep),
                                perf_mode=pmode,
                            )
                        zjunk = junkpool.tile([128, S], BF16, tag="zjunk")
                        if RELU_ENG[j2] == "A":
                            # scalar engine: relu(z + b1u), accum over s
                            nc.scalar.activation(
                                out=zjunk[:, :],
                                in_=z_ps[:, :],
                                func=act.Relu,
                                bias=b1u[:, j2 : j2 + 1],
                                accum_out=pacc[:, j2 : j2 + 1],
                            )
                        else:
                            # DVE: relu(z + c) = max(z, -c) + c, accum over s
                            nc.vector.scalar_tensor_tensor(
                                out=zjunk[:, :],
                                in0=z_ps[:, :],
                                scalar=b1n[:, j2 : j2 + 1],
                                in1=b1u[:, j2 : j2 + 1].to_broadcast([128, S]),
                                op0=mybir.AluOpType.max,
                                op1=mybir.AluOpType.add,
                                accum_out=pacc[:, j2 : j2 + 1],
                            )

                    pending.append(
                        dict(pacc=pacc, w2f=w2f, b2f=b2f, BCf=BCf, cRW=cRW, b=b)
                    )
                    flush_tail()

        flush_tail()
        if chain is not None:
            nc.vector.tensor_copy(chain[0:1, 0:1], out_acc[0:1, 0:1])
        nc.sync.dma_start(
            out=out_t[:, :].rearrange("b c -> c b"), in_=out_acc[:, :]
        )


def _prep_inputs(inputs):
    """Host-side dtype casts + re-layouts shared by all cores."""
    import ml_dtypes

    f32 = np.float32
    bf16 = ml_dtypes.bfloat16
    fp8 = ml_dtypes.float8_e4m3

    def wrap16(ids):
        """[BL, S] int -> [128, BL, S/16] int16 wrapped for dma_gather."""
        w = ids.reshape(BL, S // 16, 16).transpose(2, 0, 1).astype(np.int16)
        return np.ascontiguousarray(np.tile(w, (8, 1, 1)))

    x = np.asarray(inputs["x"]).astype(np.int32)

    # gating: pre-multiply emb through gate_w1 (and fold 1/S)
    emb = np.asarray(inputs["emb"], dtype=np.float64)
    gw1 = np.asarray(inputs["gate_w1"], dtype=np.float64)
    embg = np.ascontiguousarray((emb @ gw1) / S).astype(f32)            # [V, 256]

    exp_emb = np.clip(
        np.asarray(inputs["exp_emb"], dtype=f32) * FP8_SCALE, -240.0, 240.0
    ).astype(fp8)                                                       # [E, V, D]

    # per-core compact expert-embedding table: each core touches at most
    # BL*S = U distinct vocab ids, so (e, slot) indices fit in int16
    percore = []
    for c in range(NCORES):
        xc = x[c * BL : (c + 1) * BL]                                   # [BL, S]
        uniq, inv = np.unique(xc, return_inverse=True)
        upad = np.zeros(U, np.int64)
        upad[: uniq.size] = uniq
        ctab = np.ascontiguousarray(
            exp_emb[:, upad, :].reshape(E * U, D)
        )
        percore.append(
            dict(
                xw16=wrap16(xc),
                xws16=wrap16(inv.reshape(BL, S)),
                ctab=ctab,
            )
        )

    w1 = np.asarray(inputs["exp_w1"], dtype=f32)          # [E, D, H]
    ew1 = w1.reshape(E, DT, 128, H).transpose(0, 2, 1, 3).reshape(E * 128, DT * H)
    w1all = np.ascontiguousarray(
        np.clip(ew1 * FP8_SCALE, -240.0, 240.0)
    ).astype(fp8)
    w2 = np.asarray(inputs["exp_w2"], dtype=f32)          # [E, H, C]
    ew2 = w2.reshape(E, HT, 128, C).transpose(0, 2, 1, 3).reshape(E * 128, HT * C)
    b1 = np.asarray(inputs["exp_b1"], dtype=f32)          # [E, H]
    b1r = b1.reshape(E, HT, 128).transpose(0, 2, 1).reshape(E * 128, HT)
    b2 = np.asarray(inputs["exp_b2"], dtype=f32)          # [E, C]
    b2slot = np.zeros((E * 128, 1), f32)
    for e in range(E):
        b2slot[e * 128 : e * 128 + C, 0] = b2[e]
    w2hi = ew2.astype(bf16).astype(f32)
    w2lo = ew2 - w2hi
    wsm = np.zeros((E * 128, WSMCOLS), f32)
    wsm[:, W2COL : W2COL + HT * C] = w2hi
    wsm[:, W2LO : W2LO + HT * C] = w2lo
    wsm[:, B1COL : B1COL + HT] = b1r * (FP8_SCALE * FP8_SCALE)
    wsm[:, B2COL : B2COL + 1] = b2slot
    wsm[:, B1NEG : B1NEG + HT] = -b1r * (FP8_SCALE * FP8_SCALE)
    wsm8 = np.ascontiguousarray(wsm).astype(bf16).view(fp8)             # [E*128, 544]
    wx = np.ascontiguousarray(np.concatenate([w1all, wsm8], axis=1))    # [E*128, 8736]

    gb1 = np.ascontiguousarray(
        np.asarray(inputs["gate_b1"], dtype=f32).reshape(MT, 128).T
    )
    gw2 = np.ascontiguousarray(np.asarray(inputs["gate_w2"], dtype=f32))
    gb2r = np.ascontiguousarray(np.asarray(inputs["gate_b2"], dtype=f32).reshape(1, E))
    eyebl = np.zeros((GBL, GBL * 128), f32)
    for bl in range(GBL):
        eyebl[bl, bl * 128 : (bl + 1) * 128] = 1.0

    shared = dict(
        embg=embg, wx=wx,
        gb1=gb1, gw2=gw2, gb2r=gb2r, eyebl=eyebl,
    )
    return percore, shared


def kernel(**inputs) -> np.ndarray:
    global last_results
    if "nc" not in _compiled:
        _compiled["nc"] = build_program()
    nc = _compiled["nc"]

    percore, shared = _prep_inputs(inputs)
    in_maps = [{**percore[c], **shared} for c in range(NCORES)]
    trace = os.environ.get("KERNEL_TRACE", "0") == "1"
    kw = {}
    if trace:
        tdir = os.environ.get("KERNEL_TRACE_DIR", "/root/problem/trace_out")
        os.makedirs(tdir, exist_ok=True)
        kw = dict(trace=True, tmpdir=tdir)
    res = run_bass_kernel_spmd(nc, in_maps, list(range(NCORES)), **kw)
    last_results = res
    out = np.concatenate([res.results[c]["out"] for c in range(NCORES)], axis=0)
    return np.ascontiguousarray(out.astype(np.float32))


# revision 87
# speedup vs baseline: 1.2239x; 1.2239x over previous
"""Trainium2 Bass kernel for an MoE routing module.

Strategy: data-parallel over the batch — each of the 8 NeuronCores runs the
full pipeline (gating -> top-2 -> expert MLPs) for its 8 samples. All
data-dependent expert selection happens on device via gathers driven by the
top-2 result; there are no collectives and no registers.

Host-side prep:
  - gating embedding is pre-multiplied through the gate's first layer:
    embG = (emb @ gate_w1) / S (f64 accumulate, f32 store), so the device
    gathers 256-wide rows instead of 1024-wide ones and the gate L1 matmul
    disappears. Top-2 selection needs exact-ish f32 logits (margins are
    ~1e-5), so embG stays f32 and the tiny L2 matmul runs in true f32.
  - per-core COMPACT expert-embedding table: a core touches at most
    BL*S = 4096 distinct vocab ids, so the host dedupes them and ships
    [E*4096, D] in e4m3 (*FP8_SCALE). (e, slot) indices then fit int16,
    which lets ONE gpsimd dma_gather fetch all 512 token rows per (b,k)
    (vs 4 indirect DMAs whose SWDGE descriptor prep made the Pool engine
    the pacing engine).
  - expert weights are packed per-expert into ONE fp8 "mega table"
    [E*128, WXCOLS]: W1 as e4m3*FP8_SCALE (t-major d-tiles), then the raw
    bytes of a bf16 side table (W2 hi+lo pair so W2 reconstructs to ~f32,
    b1 pre-scaled into the unscaled-z domain, b2). A [128,1] index tile
    (value e*128+p) gathers everything for an expert in a single indirect
    DMA with 128 fat descriptors.
Expert math: tok/W1 fp8 with DoubleRow matmuls (K=256 per instr, fp32 PSUM).
Tokens arrive ALREADY TRANSPOSED from dma_gather(transpose=True): the DMA
transposes at u16 granularity, so the host byte-permutes each table row to
make u16 unit (jj*128+p) hold the fp8 pair (d, d+512) with d = jj*128+p;
the DoubleRow rhs then reads [128, l(stride 1), token(stride 2)] and W1 is
host-packed to the same (jj, l) order. This removes the PE identity-matmul
transposes and all psum->SBUF copies that previously paced the kernel.
RELU_ENG balances the per-[128,512]-tile relu+accum between the scalar and
vector engines. The FP8_SCALE^-2 and 1/S factors fold into the pooled-
vector scale together with the routing weight; b2 and the routing weight
fold into one scalar_tensor_tensor op.
The gating chain is kept short (it gates the pipelined expert loop): h^T is
produced directly in partition layout by N=1 partition-sum matmuls, gb2 is
a rank-1 matmul into the logit psum, top-2 reads the logit-transpose psum
directly, and the per-(b,k) scalars are broadcast to all partitions with
one-hot-row lhsT matmuls instead of a DRAM bounce.

HW gotchas (verified on device): indirect DMA consumes exactly ONE index
per destination partition; walrus rejects DVE tensor_tensor with two PSUM
operands, f32r matmul inputs that aren't produced as f32r, and scale+bias+
accum_out all on one activation (runtime failure).
"""

import os
import sys

for _p in ("/opt/trn_rl_repo", "/root/.axon_site/_ro/trn_rl_repo"):
    if os.path.isdir(_p) and _p not in sys.path:
        sys.path.insert(0, _p)

import numpy as np

import concourse.bacc as bacc
import concourse.tile as tile
import concourse.mybir as mybir
from concourse.bass import IndirectOffsetOnAxis
from concourse.bass_utils import run_bass_kernel_spmd
from concourse.masks import make_identity

F32 = mybir.dt.float32
F32R = mybir.dt.float32r
BF16 = mybir.dt.bfloat16
F8 = mybir.dt.float8e4
I32 = mybir.dt.int32
I16 = mybir.dt.int16
U32 = mybir.dt.uint32

V, D, H, E, C, TOPK = 16000, 1024, 1024, 8, 16, 2
B, S = 64, 512
GATE_H = 256
NCORES = 8
BL = B // NCORES          # samples per core
DT = D // 128             # 8 d-tiles
HT = H // 128             # 8 h-tiles
ST = S // 128             # 4 s-tiles
MT = GATE_H // 128        # 2 gate-hidden tiles
NGRP = 4                  # sample groups per core (pipelining)
GBL = BL // NGRP          # samples per group

RELU_ENG = "AADDAADD"     # per h-tile: A=scalar, D=DVE relu+accum engine
W2F_ENG = "D"             # engine for the W2 hi+lo add (D=DVE, P=gpsimd)
U = 4096                  # per-core compact vocab (8 samples x 512 tokens)

# fp8 scaling: tok and W1 stored as e4m3 * FP8_SCALE; z_psum carries
# FP8_SCALE^2, divided out in the pooled-vector scale.
FP8_SCALE = 64.0
FP8_UNSCALE = 1.0 / (FP8_SCALE * FP8_SCALE)

# bf16 side table layout (within the fp8 mega table, bytes after W1)
W2COL = 0                 # W2 hi [HT*C]
W2LO = W2COL + HT * C     # 128   W2 lo [HT*C]
B1COL = W2LO + HT * C     # 256   b1 * FP8_SCALE^2 [HT]
B1NEG = B1COL + HT        # 264   -b1 * FP8_SCALE^2 [HT]
B2COL = B1NEG + HT        # 272   b2 (partitions 0..C-1) [1]
WSMCOLS = 288             # padded bf16 row length
W1OFF = DT * H            # 8192 fp8 bytes of W1
WXCOLS = W1OFF + 2 * WSMCOLS  # 8768 fp8 row length of the mega table

_compiled = {}
last_results = None       # BassKernelResults of the most recent run (for test.py)


def build_program(reps=1):
    """reps>1 repeats the whole compute body (benchmarking aid)."""
    nc = bacc.Bacc("TRN2", target_bir_lowering=False, debug=False, num_devices=NCORES)
    act = mybir.ActivationFunctionType

    xw_t = nc.dram_tensor("xw16", [128, BL, S // 16], I16, kind="ExternalInput")
    xws_t = nc.dram_tensor("xws16", [128, BL, S // 16], I16, kind="ExternalInput")
    embg_t = nc.dram_tensor("embg", [V, GATE_H], F32, kind="ExternalInput")
    ctab_t = nc.dram_tensor("ctab", [E * U, D], F8, kind="ExternalInput")
    wx_t = nc.dram_tensor("wx", [E * 128, WXCOLS], F8, kind="ExternalInput")
    gb1_t = nc.dram_tensor("gb1", [128, MT], F32, kind="ExternalInput")
    gw2_t = nc.dram_tensor("gw2", [GATE_H, E], F32, kind="ExternalInput")
    gb2_t = nc.dram_tensor("gb2r", [1, E], F32, kind="ExternalInput")
    eyebl_t = nc.dram_tensor("eyebl", [GBL, GBL * 128], F32, kind="ExternalInput")
    out_t = nc.dram_tensor("out", [BL, C], F32, kind="ExternalOutput")

    with tile.TileContext(nc) as tc:
        with (
            tc.tile_pool(name="const", bufs=1) as cpool,
            tc.tile_pool(name="dram", bufs=1, space="DRAM") as dpool,
        ):
            # ---- constants ----
            id_f = cpool.tile([128, 128], F32)
            make_identity(nc, id_f[:, :])
            ones_k = cpool.tile([128, 1], F32)      # lhsT for partition-sum MMs
            nc.vector.memset(ones_k[:, :], 1.0)

            ones_m = cpool.tile([1, 128], F32)      # lhsT for K=1 broadcast MMs
            nc.vector.memset(ones_m[:, :], 1.0)
            iota_p = cpool.tile([128, 1], I32)      # value = partition index
            nc.gpsimd.iota(iota_p[:, :], pattern=[[0, 1]], base=0, channel_multiplier=1)
            zero_c = cpool.tile([128, 1], F32)
            nc.vector.memset(zero_c[:, :], 0.0)

            # int16 wrapped indices for dma_gather (pre-wrapped on host):
            # xw = raw vocab ids (gating), xws = compact slot ids (experts)
            xw = cpool.tile([128, BL, S // 16], I16)
            nc.sync.dma_start(out=xw[:, :, :], in_=xw_t[:, :, :])
            xws = cpool.tile([128, BL, S // 16], I16)
            nc.sync.dma_start(out=xws[:, :, :], in_=xws_t[:, :, :])

            gb1_sb = cpool.tile([128, MT], F32)
            nc.sync.dma_start(out=gb1_sb[:, :], in_=gb1_t[:, :])
            gb2_sb = cpool.tile([1, E], F32)
            nc.sync.dma_start(out=gb2_sb[:, :], in_=gb2_t[:, :])
            eyebl_sb = cpool.tile([GBL, GBL * 128], F32)
            nc.sync.dma_start(out=eyebl_sb[:, :], in_=eyebl_t[:, :])
            gw2_sb = cpool.tile([128, MT, E], F32)
            nc.sync.dma_start(
                out=gw2_sb[:, :, :], in_=gw2_t[:, :].rearrange("(m p) e -> p m e", p=128)
            )

            consts = dict(
                id_f=id_f, ones_k=ones_k,
                ones_m=ones_m, iota_p=iota_p, zero_c=zero_c, xw=xw, xws=xws,
                gb1_sb=gb1_sb, gb2_sb=gb2_sb, gw2_sb=gw2_sb, eyebl_sb=eyebl_sb,
            )
            tensors = dict(
                embg_t=embg_t, ctab_t=ctab_t, wx_t=wx_t, out_t=out_t,
            )
            # chain tile serializes reps so the benchmark differential is honest
            chain = None
            if reps > 1:
                chain = cpool.tile([1, 1], F32)
                nc.vector.memset(chain[:, :], 0.0)
            for rep in range(reps):
                _body_once(nc, tc, act, rep, dpool, consts, tensors, chain)

    nc.compile()
    return nc


def _body_once(nc, tc, act, rep, dpool, cn, tn, chain=None):
    sfx = f"_r{rep}"
    id_f = cn["id_f"]
    ones_k, ones_m, iota_p, zero_c = cn["ones_k"], cn["ones_m"], cn["iota_p"], cn["zero_c"]
    xw, xws = cn["xw"], cn["xws"]
    gb1_sb, gb2_sb, gw2_sb = cn["gb1_sb"], cn["gb2_sb"], cn["gw2_sb"]
    eyebl_sb = cn["eyebl_sb"]
    embg_t, ctab_t, wx_t, out_t = tn["embg_t"], tn["ctab_t"], tn["wx_t"], tn["out_t"]

    with (
        tc.tile_pool(name=f"persist{sfx}", bufs=1) as ppool,
        tc.tile_pool(name=f"bc{sfx}", bufs=2) as bcpool,
        # gating pools
        tc.tile_pool(name=f"gat{sfx}", bufs=4) as gpool,
        tc.tile_pool(name=f"gsb{sfx}", bufs=2) as gspool,
        tc.tile_pool(name=f"gps{sfx}", bufs=1, space="PSUM") as gps,
        tc.tile_pool(name=f"gpss{sfx}", bufs=2, space="PSUM") as gps_s,
        # expert pools
        tc.tile_pool(name=f"exi{sfx}", bufs=3) as xipool,
        tc.tile_pool(name=f"etok{sfx}", bufs=3) as tokpool,
        tc.tile_pool(name=f"ew{sfx}", bufs=3) as wpool,
        tc.tile_pool(name=f"esm{sfx}", bufs=3) as smpool,
        tc.tile_pool(name=f"ejunk{sfx}", bufs=2) as junkpool,
        tc.tile_pool(name=f"epsz{sfx}", bufs=4, space="PSUM") as eps_z,
        tc.tile_pool(name=f"epso{sfx}", bufs=1, space="PSUM") as eps_o,
    ):
        out_acc = ppool.tile([C, BL], F32)
        nc.vector.memset(out_acc[:, :], 0.0)

        # deferred W2 tail of the previous (b,k): emitting it after the next
        # (b,k)'s GEMMs keeps the in-order PE queue from stalling on the
        # relu->psc dependency
        pending = []

        def flush_tail():
            if not pending:
                return
            st = pending.pop()
            psc = smpool.tile([128, HT], F32, tag="psc")
            nc.vector.scalar_tensor_tensor(
                out=psc[:, :],
                in0=st["pacc"][:, :],
                scalar=FP8_UNSCALE / S,
                in1=st["BCf"][:, st["cRW"] : st["cRW"] + 1].to_broadcast([128, HT]),
                op0=mybir.AluOpType.mult,
                op1=mybir.AluOpType.mult,
            )
            eo_ps = eps_o.tile([C, 1], F32, tag="eo")
            for j2 in range(HT):
                nc.tensor.matmul(
                    out=eo_ps[:, :],
                    lhsT=st["w2f"][:, j2 * C : (j2 + 1) * C],
                    rhs=psc[:, j2 : j2 + 1],
                    start=(j2 == 0),
                    stop=(j2 == HT - 1),
                )
            # out_acc[:, b] += rw*(p@W2) + rw*b2: psc already carries rw, so
            # add rw*b2 via stt: (b2 mult rw) add eo
            eo2 = smpool.tile([C, 1], F32, tag="eo2")
            nc.vector.scalar_tensor_tensor(
                out=eo2[:, :],
                in0=st["b2f"][:, :],
                scalar=st["BCf"][0:C, st["cRW"] : st["cRW"] + 1],
                in1=eo_ps[:, :],
                op0=mybir.AluOpType.mult,
                op1=mybir.AluOpType.add,
            )
            b = st["b"]
            nc.vector.tensor_add(
                out_acc[:, b : b + 1], out_acc[:, b : b + 1], eo2[:, :]
            )

        for g in range(NGRP):
            b0 = g * GBL
            # ============ gating for samples [b0, b0+GBL) (f32) ============
            # hT[p, m] = relu(pooled @ gw1 + gb1)[m*128+p], computed directly
            # in partition layout: 8 tiny N=1 matmuls sum gtok g-slices over
            # tokens (f32r; m13 truncation is ~1e-8 on the logits, margins
            # are ~1e-5).
            hTs = gspool.tile([128, MT, GBL], F32, tag="hTs")
            for bl in range(GBL):
                b = b0 + bl
                # embG rows for this sample's tokens: [128, ST, 256]
                gtok = gpool.tile([128, ST, GATE_H], F32, tag="gtok")
                nc.gpsimd.dma_gather(
                    out_ap=gtok[:, :, :],
                    in_ap=embg_t[:, :],
                    idxs_ap=xw[:, b, :],
                    num_idxs=S,
                    num_idxs_reg=S,
                    elem_size=GATE_H,
                    transpose=False,
                )
                hp = gps.tile([128, MT], F32, tag="pp")
                for m in range(MT):
                    for t in range(ST):
                        nc.tensor.matmul(
                            out=hp[:, m : m + 1],
                            lhsT=gtok[:, t, m * 128 : (m + 1) * 128],
                            rhs=ones_k[:, :],
                            start=(t == 0),
                            stop=(t == ST - 1),
                        )
                # h = relu(hp + gb1)  (1/S is folded into embG on host)
                aT = gspool.tile([128, MT], F32, tag="aT")
                nc.vector.tensor_add(aT[:, :], hp[:, :], gb1_sb[:, :])
                nc.vector.tensor_scalar_max(hTs[:, :, bl], aT[:, :], 0.0)

            # gate layer 2 + gb2 (rank-1 matmul) -> logits [e, b], then
            # transpose to [b, e]; all on PE so the chain stays short
            l_ps = gps_s.tile([E, GBL], F32, tag="gmisc")
            for m in range(MT):
                nc.tensor.matmul(
                    out=l_ps[:, :],
                    lhsT=gw2_sb[:, m, :],
                    rhs=hTs[:, m, :],
                    start=(m == 0),
                    stop=False,
                )
            nc.tensor.matmul(
                out=l_ps[:, :],
                lhsT=gb2_sb[0:1, :],
                rhs=ones_m[0:1, 0:GBL],
                start=False,
                stop=True,
            )
            l_sb = gspool.tile([E, GBL], F32, tag="l_sb")
            nc.vector.tensor_copy(l_sb[:, :], l_ps[:, :])
            lt_ps = gps_s.tile([GBL, E], F32, tag="gmisc")
            nc.tensor.matmul(
                out=lt_ps[:, :], lhsT=l_sb[:, :], rhs=id_f[0:E, 0:E],
                start=True, stop=True,
            )

            # top-2 of logits == top-2 of softmax (monotone); DVE reads the
            # psum tile directly
            mx = gspool.tile([GBL, 8], F32, tag="mx")
            mi = gspool.tile([GBL, 8], U32, tag="mi")
            nc.vector.max_with_indices(mx[:, :], mi[:, :], lt_ps[:, :])

            # renormalized top-2 softmax weights:
            # rw1 = 1/(1+exp(l2-l1)), rw2 = exp(l2-l1)/(1+exp(l2-l1))
            dlt = gspool.tile([GBL, 1], F32, tag="dlt")
            nc.vector.tensor_sub(dlt[:, :], mx[:, 1:2], mx[:, 0:1])
            q = gspool.tile([GBL, 1], F32, tag="q")
            nc.scalar.activation(out=q[:, :], in_=dlt[:, :], func=act.Exp)
            sden = gspool.tile([GBL, 1], F32, tag="sden")
            nc.vector.tensor_scalar_add(sden[:, :], q[:, :], 1.0)
            rw1 = gspool.tile([GBL, 1], F32, tag="rw1")
            nc.vector.reciprocal(rw1[:, :], sden[:, :])
            rw2 = gspool.tile([GBL, 1], F32, tag="rw2")
            nc.vector.tensor_mul(rw2[:, :], q[:, :], rw1[:, :])

            # pack per-(b,k) scalars: cols bl*8 + {0,1}=e*U, {2,3}=e*128,
            # {6,7}=rw ({4,5} unused)
            ei_f = gspool.tile([GBL, TOPK], F32, tag="ei_f")
            nc.vector.tensor_copy(ei_f[:, :], mi[:, 0:TOPK])
            vals = gspool.tile([GBL, 8], F32, tag="vals")
            nc.vector.tensor_scalar_mul(vals[:, 0:2], ei_f[:, :], float(U))
            nc.vector.tensor_scalar_mul(vals[:, 2:4], ei_f[:, :], 128.0)
            nc.vector.tensor_scalar_mul(vals[:, 4:6], ei_f[:, :], 0.0)
            nc.vector.tensor_copy(vals[:, 6:7], rw1[:, :])
            nc.vector.tensor_copy(vals[:, 7:8], rw2[:, :])

            # broadcast vals[bl, :] to all partitions of cols bl*8..bl*8+8
            # via one-hot-row lhsT matmuls (no DRAM bounce)
            if chain is not None:
                # unused col 4: forces rep r to wait on rep r-1's result
                nc.vector.tensor_copy(vals[0:1, 4:5], chain[0:1, 0:1])
            bc_ps = gps_s.tile([128, GBL * 8], F32, tag="gmisc")
            for bl in range(GBL):
                nc.tensor.matmul(
                    out=bc_ps[:, bl * 8 : (bl + 1) * 8],
                    lhsT=eyebl_sb[:, bl * 128 : (bl + 1) * 128],
                    rhs=vals[:, :],
                    start=True,
                    stop=True,
                )
            BCf = bcpool.tile([128, GBL * 8], F32, tag="bcf")
            BCi = bcpool.tile([128, GBL * 8], I32, tag="bci")
            BCi16 = bcpool.tile([128, GBL * 8], I16, tag="bci16")
            nc.vector.tensor_copy(BCf[:, :], bc_ps[:, :])
            nc.vector.tensor_copy(BCi[:, :], bc_ps[:, :])    # cast f32->i32
            nc.vector.tensor_copy(BCi16[:, :], bc_ps[:, :])  # cast f32->i16

            # ============ experts for this group (fp8) ============
            for bl in range(GBL):
                b = b0 + bl
                for k in range(TOPK):
                    cEV = bl * 8 + k
                    cE128 = bl * 8 + 2 + k
                    cRW = bl * 8 + 6 + k

                    # compact-table indices: slot + e*U (fits int16: <= 32763)
                    tok_idx = xipool.tile([128, S // 16], I16, tag="tok_idx")
                    nc.vector.tensor_add(
                        tok_idx[:, :],
                        xws[:, b, :],
                        BCi16[:, cEV : cEV + 1].to_broadcast([128, S // 16]),
                    )
                    w_idx = xipool.tile([128, 1], I32, tag="w_idx")
                    nc.vector.tensor_add(
                        w_idx[:, :], iota_p[:, :], BCi[:, cE128 : cE128 + 1]
                    )

                    # transposed gather (u16 granularity is exact in the i16
                    # view): tok16[p, jj, i] = row16_i[jj*128 + p]. The host
                    # byte-permutes each table row so u16 unit (jj*128+p)
                    # holds the fp8 pair (d, d+512) with d = jj*128+p — the
                    # DoubleRow rhs then reads [p, l(stride 1), i(stride 2)].
                    tok16 = tokpool.tile([128, ST, S], I16, tag="tok")
                    nc.gpsimd.dma_gather(
                        out_ap=tok16[:, :, :],
                        in_ap=ctab_t[:, :].bitcast(I16),
                        idxs_ap=tok_idx[:, :],
                        num_idxs=S,
                        num_idxs_reg=S,
                        elem_size=D // 2,
                        transpose=True,
                    )
                    # one gather for W1 (fp8) + bf16 side table (as raw bytes)
                    wg = wpool.tile([128, WXCOLS], F8, tag="wg")
                    nc.gpsimd.indirect_dma_start(
                        out=wg[:, :],
                        out_offset=None,
                        in_=wx_t[:, :],
                        in_offset=IndirectOffsetOnAxis(ap=w_idx[:, :], axis=0),
                    )
                    wsm = wg[:, W1OFF:].bitcast(BF16)      # [128, WSMCOLS] bf16
                    b1un = smpool.tile([128, 2 * HT], F32, tag="b1un")
                    nc.vector.tensor_copy(b1un[:, :], wsm[:, B1COL : B1COL + 2 * HT])
                    b1u = b1un[:, 0:HT]
                    b1n = b1un[:, HT : 2 * HT]
                    b2f = smpool.tile([C, 1], F32, tag="b2f")
                    nc.vector.tensor_copy(b2f[:, :], wsm[0:C, B2COL : B2COL + 1])
                    w2f = smpool.tile([128, HT * C], F32, tag="w2f")
                    w2f_eng = nc.gpsimd if W2F_ENG == "P" else nc.vector
                    w2f_eng.tensor_add(
                        w2f[:, :], wsm[:, W2COL : W2COL + HT * C],
                        wsm[:, W2LO : W2LO + HT * C],
                    )

                    # z[h_tile] = relu(tokT.T @ W1 + b1*SC^2); accumulate sum
                    # over s. fp8 DoubleRow contracts the (d, d+512) pair of
                    # each u16 unit per matmul; W1 is host-packed to match.
                    w1v = wg[:, 0:W1OFF].rearrange("p (kk h) -> p kk h", kk=DT)
                    pacc = smpool.tile([128, HT], F32, tag="pacc")
                    for j2 in range(HT):
                        z_ps = eps_z.tile([128, S], F32, tag="z")
                        for jj in range(ST):
                            rhs = (
                                tok16[:, jj, :].bitcast(F8)
                                .rearrange("p (i l) -> p l i", l=2)
                            )
                            nc.tensor.matmul(
                                out=z_ps[:, :],
                                lhsT=w1v[:, 2 * jj : 2 * jj + 2,
                                         j2 * 128 : (j2 + 1) * 128],
                                rhs=rhs,
                                start=(jj == 0),
                                stop=(jj == ST - 1),
                                perf_mode=mybir.MatmulPerfMode.DoubleRow,
                            )
                        zjunk = junkpool.tile([128, S], BF16, tag="zjunk")
                        if RELU_ENG[j2] == "A":
                            # scalar engine: relu(z + b1u), accum over s
                            nc.scalar.activation(
                                out=zjunk[:, :],
                                in_=z_ps[:, :],
                                func=act.Relu,
                                bias=b1u[:, j2 : j2 + 1],
                                accum_out=pacc[:, j2 : j2 + 1],
                            )
                        else:
                            # DVE: relu(z + c) = max(z, -c) + c, accum over s
                            nc.vector.scalar_tensor_tensor(
                                out=zjunk[:, :],
                                in0=z_ps[:, :],
                                scalar=b1n[:, j2 : j2 + 1],
                                in1=b1u[:, j2 : j2 + 1].to_broadcast([128, S]),
                                op0=mybir.AluOpType.max,
                                op1=mybir.AluOpType.add,
                                accum_out=pacc[:, j2 : j2 + 1],
                            )

                    pending.append(
                        dict(pacc=pacc, w2f=w2f, b2f=b2f, BCf=BCf, cRW=cRW, b=b)
                    )
                    flush_tail()

        flush_tail()
        if chain is not None:
            nc.vector.tensor_copy(chain[0:1, 0:1], out_acc[0:1, 0:1])
        nc.sync.dma_start(
            out=out_t[:, :].rearrange("b c -> c b"), in_=out_acc[:, :]
        )


def _prep_inputs(inputs):
    """Host-side dtype casts + re-layouts shared by all cores."""
    import ml_dtypes

    f32 = np.float32
    bf16 = ml_dtypes.bfloat16
    fp8 = ml_dtypes.float8_e4m3

    def wrap16(ids):
        """[BL, S] int -> [128, BL, S/16] int16 wrapped for dma_gather."""
        w = ids.reshape(BL, S // 16, 16).transpose(2, 0, 1).astype(np.int16)
        return np.ascontiguousarray(np.tile(w, (8, 1, 1)))

    x = np.asarray(inputs["x"]).astype(np.int32)

    # gating: pre-multiply emb through gate_w1 (and fold 1/S)
    emb = np.asarray(inputs["emb"], dtype=np.float64)
    gw1 = np.asarray(inputs["gate_w1"], dtype=np.float64)
    embg = np.ascontiguousarray((emb @ gw1) / S).astype(f32)            # [V, 256]

    exp_emb = np.clip(
        np.asarray(inputs["exp_emb"], dtype=f32) * FP8_SCALE, -240.0, 240.0
    ).astype(fp8)                                                       # [E, V, D]
    # byte-permute each row so the u16-granularity transposed gather lands
    # d-major: stored u16 unit (jj*128+p) = fp8 pair (d, d+512), d=jj*128+p
    beta = np.arange(D)
    u, lo = beta // 2, beta % 2
    dperm = lo * 512 + (u // 128) * 128 + (u % 128)
    exp_emb = np.ascontiguousarray(exp_emb[:, :, dperm])

    # per-core compact expert-embedding table: each core touches at most
    # BL*S = U distinct vocab ids, so (e, slot) indices fit in int16
    percore = []
    for c in range(NCORES):
        xc = x[c * BL : (c + 1) * BL]                                   # [BL, S]
        uniq, inv = np.unique(xc, return_inverse=True)
        upad = np.zeros(U, np.int64)
        upad[: uniq.size] = uniq
        ctab = np.ascontiguousarray(
            exp_emb[:, upad, :].reshape(E * U, D)
        )
        percore.append(
            dict(
                xw16=wrap16(xc),
                xws16=wrap16(inv.reshape(BL, S)),
                ctab=ctab,
            )
        )

    # W1 packed to match the pair layout: col (jj*2 + l)*H + h on partition p
    # holds W1[l*512 + jj*128 + p, h]
    w1 = np.asarray(inputs["exp_w1"], dtype=f32)          # [E, D, H]
    ew1 = (
        w1.reshape(E, 2, ST, 128, H)                      # [e, l, jj, p, h]
        .transpose(0, 3, 2, 1, 4)                         # [e, p, jj, l, h]
        .reshape(E * 128, DT * H)
    )
    w1all = np.ascontiguousarray(
        np.clip(ew1 * FP8_SCALE, -240.0, 240.0)
    ).astype(fp8)
    w2 = np.asarray(inputs["exp_w2"], dtype=f32)          # [E, H, C]
    ew2 = w2.reshape(E, HT, 128, C).transpose(0, 2, 1, 3).reshape(E * 128, HT * C)
    b1 = np.asarray(inputs["exp_b1"], dtype=f32)          # [E, H]
    b1r = b1.reshape(E, HT, 128).transpose(0, 2, 1).reshape(E * 128, HT)
    b2 = np.asarray(inputs["exp_b2"], dtype=f32)          # [E, C]
    b2slot = np.zeros((E * 128, 1), f32)
    for e in range(E):
        b2slot[e * 128 : e * 128 + C, 0] = b2[e]
    w2hi = ew2.astype(bf16).astype(f32)
    w2lo = ew2 - w2hi
    wsm = np.zeros((E * 128, WSMCOLS), f32)
    wsm[:, W2COL : W2COL + HT * C] = w2hi
    wsm[:, W2LO : W2LO + HT * C] = w2lo
    wsm[:, B1COL : B1COL + HT] = b1r * (FP8_SCALE * FP8_SCALE)
    wsm[:, B2COL : B2COL + 1] = b2slot
    wsm[:, B1NEG : B1NEG + HT] = -b1r * (FP8_SCALE * FP8_SCALE)
    wsm8 = np.ascontiguousarray(wsm).astype(bf16).view(fp8)             # [E*128, 544]
    wx = np.ascontiguousarray(np.concatenate([w1all, wsm8], axis=1))    # [E*128, 8736]

    gb1 = np.ascontiguousarray(
        np.asarray(inputs["gate_b1"], dtype=f32).reshape(MT, 128).T
    )
    gw2 = np.ascontiguousarray(np.asarray(inputs["gate_w2"], dtype=f32))
    gb2r = np.ascontiguousarray(np.asarray(inputs["gate_b2"], dtype=f32).reshape(1, E))
    eyebl = np.zeros((GBL, GBL * 128), f32)
    for bl in range(GBL):
        eyebl[bl, bl * 128 : (bl + 1) * 128] = 1.0

    shared = dict(
        embg=embg, wx=wx,
        gb1=gb1, gw2=gw2, gb2r=gb2r, eyebl=eyebl,
    )
    return percore, shared


def kernel(**inputs) -> np.ndarray:
    global last_results
    if "nc" not in _compiled:
        _compiled["nc"] = build_program()
    nc = _compiled["nc"]

    percore, shared = _prep_inputs(inputs)
    in_maps = [{**percore[c], **shared} for c in range(NCORES)]
    trace = os.environ.get("KERNEL_TRACE", "0") == "1"
    kw = {}
    if trace:
        tdir = os.environ.get("KERNEL_TRACE_DIR", "/root/problem/trace_out")
        os.makedirs(tdir, exist_ok=True)
        kw = dict(trace=True, tmpdir=tdir)
    res = run_bass_kernel_spmd(nc, in_maps, list(range(NCORES)), **kw)
    last_results = res
    out = np.concatenate([res.results[c]["out"] for c in range(NCORES)], axis=0)
    return np.ascontiguousarray(out.astype(np.float32))


# revision 88
# speedup vs baseline: 1.2419x; 1.0147x over previous
"""Trainium2 Bass kernel for an MoE routing module.

Strategy: data-parallel over the batch — each of the 8 NeuronCores runs the
full pipeline (gating -> top-2 -> expert MLPs) for its 8 samples. All
data-dependent expert selection happens on device via gathers driven by the
top-2 result; there are no collectives and no registers.

Host-side prep:
  - gating embedding is pre-multiplied through the gate's first layer:
    embG = (emb @ gate_w1) / S (f64 accumulate, f32 store), so the device
    gathers 256-wide rows instead of 1024-wide ones and the gate L1 matmul
    disappears. Top-2 selection needs exact-ish f32 logits (margins are
    ~1e-5), so embG stays f32 and the tiny L2 matmul runs in true f32.
  - per-core COMPACT expert-embedding table: a core touches at most
    BL*S = 4096 distinct vocab ids, so the host dedupes them and ships
    [E*4096, D] in e4m3 (*FP8_SCALE). (e, slot) indices then fit int16,
    which lets ONE gpsimd dma_gather fetch all 512 token rows per (b,k)
    (vs 4 indirect DMAs whose SWDGE descriptor prep made the Pool engine
    the pacing engine).
  - expert weights are packed per-expert into ONE fp8 "mega table"
    [E*128, WXCOLS]: W1 as e4m3*FP8_SCALE (t-major d-tiles), then the raw
    bytes of a bf16 side table (W2 hi+lo pair so W2 reconstructs to ~f32,
    b1 pre-scaled into the unscaled-z domain, b2). A [128,1] index tile
    (value e*128+p) gathers everything for an expert in a single indirect
    DMA with 128 fat descriptors.
Expert math: tok/W1 fp8 with DoubleRow matmuls (K=256 per instr, fp32 PSUM).
Tokens arrive ALREADY TRANSPOSED from dma_gather(transpose=True): the DMA
transposes at u16 granularity, so the host byte-permutes each table row to
make u16 unit (jj*128+p) hold the fp8 pair (d, d+512) with d = jj*128+p;
the DoubleRow rhs then reads [128, l(stride 1), token(stride 2)] and W1 is
host-packed to the same (jj, l) order. This removes the PE identity-matmul
transposes and all psum->SBUF copies that previously paced the kernel.
RELU_ENG balances the per-[128,512]-tile relu+accum between the scalar and
vector engines. The FP8_SCALE^-2 and 1/S factors fold into the pooled-
vector scale together with the routing weight; b2 and the routing weight
fold into one scalar_tensor_tensor op.
The gating chain is kept short (it gates the pipelined expert loop): h^T is
produced directly in partition layout by N=1 partition-sum matmuls, gb2 is
a rank-1 matmul into the logit psum, top-2 reads the logit-transpose psum
directly, and the per-(b,k) scalars are broadcast to all partitions with
one-hot-row lhsT matmuls instead of a DRAM bounce.

HW gotchas (verified on device): indirect DMA consumes exactly ONE index
per destination partition; walrus rejects DVE tensor_tensor with two PSUM
operands, f32r matmul inputs that aren't produced as f32r, and scale+bias+
accum_out all on one activation (runtime failure).
"""

import os
import sys

for _p in ("/opt/trn_rl_repo", "/root/.axon_site/_ro/trn_rl_repo"):
    if os.path.isdir(_p) and _p not in sys.path:
        sys.path.insert(0, _p)

import numpy as np

import concourse.bacc as bacc
import concourse.tile as tile
import concourse.mybir as mybir
from concourse.bass import IndirectOffsetOnAxis
from concourse.bass_utils import run_bass_kernel_spmd
from concourse.masks import make_identity

F32 = mybir.dt.float32
F32R = mybir.dt.float32r
BF16 = mybir.dt.bfloat16
F8 = mybir.dt.float8e4
I32 = mybir.dt.int32
I16 = mybir.dt.int16
U32 = mybir.dt.uint32

V, D, H, E, C, TOPK = 16000, 1024, 1024, 8, 16, 2
B, S = 64, 512
GATE_H = 256
NCORES = 8
BL = B // NCORES          # samples per core
DT = D // 128             # 8 d-tiles
HT = H // 128             # 8 h-tiles
ST = S // 128             # 4 s-tiles
MT = GATE_H // 128        # 2 gate-hidden tiles
NGRP = 4                  # sample groups per core (pipelining)
GBL = BL // NGRP          # samples per group

RELU_ENG = "AADDAADA"     # per h-tile: A=scalar, D=DVE relu+accum engine
W2F_ENG = "D"             # engine for the W2 hi+lo add (D=DVE, P=gpsimd)
U = 4096                  # per-core compact vocab (8 samples x 512 tokens)

# fp8 scaling: tok and W1 stored as e4m3 * FP8_SCALE; z_psum carries
# FP8_SCALE^2, divided out in the pooled-vector scale.
FP8_SCALE = 64.0
FP8_UNSCALE = 1.0 / (FP8_SCALE * FP8_SCALE)

# bf16 side table layout (within the fp8 mega table, bytes after W1)
W2COL = 0                 # W2 hi [HT*C]
W2LO = W2COL + HT * C     # 128   W2 lo [HT*C]
B1COL = W2LO + HT * C     # 256   b1 * FP8_SCALE^2 [HT]
B1NEG = B1COL + HT        # 264   -b1 * FP8_SCALE^2 [HT]
B2COL = B1NEG + HT        # 272   b2 (partitions 0..C-1) [1]
WSMCOLS = 288             # padded bf16 row length
W1OFF = DT * H            # 8192 fp8 bytes of W1
WXCOLS = W1OFF + 2 * WSMCOLS  # 8768 fp8 row length of the mega table

_compiled = {}
last_results = None       # BassKernelResults of the most recent run (for test.py)


def build_program(reps=1):
    """reps>1 repeats the whole compute body (benchmarking aid)."""
    nc = bacc.Bacc("TRN2", target_bir_lowering=False, debug=False, num_devices=NCORES)
    act = mybir.ActivationFunctionType

    xw_t = nc.dram_tensor("xw16", [128, BL, S // 16], I16, kind="ExternalInput")
    xws_t = nc.dram_tensor("xws16", [128, BL, S // 16], I16, kind="ExternalInput")
    embg_t = nc.dram_tensor("embg", [V, GATE_H], F32, kind="ExternalInput")
    ctab_t = nc.dram_tensor("ctab", [E * U, D], F8, kind="ExternalInput")
    wx_t = nc.dram_tensor("wx", [E * 128, WXCOLS], F8, kind="ExternalInput")
    gb1_t = nc.dram_tensor("gb1", [128, MT], F32, kind="ExternalInput")
    gw2_t = nc.dram_tensor("gw2", [GATE_H, E], F32, kind="ExternalInput")
    gb2_t = nc.dram_tensor("gb2r", [1, E], F32, kind="ExternalInput")
    eyebl_t = nc.dram_tensor("eyebl", [GBL, GBL * 128], F32, kind="ExternalInput")
    out_t = nc.dram_tensor("out", [BL, C], F32, kind="ExternalOutput")

    with tile.TileContext(nc) as tc:
        with (
            tc.tile_pool(name="const", bufs=1) as cpool,
            tc.tile_pool(name="dram", bufs=1, space="DRAM") as dpool,
        ):
            # ---- constants ----
            id_f = cpool.tile([128, 128], F32)
            make_identity(nc, id_f[:, :])
            ones_k = cpool.tile([128, 1], F32)      # lhsT for partition-sum MMs
            nc.vector.memset(ones_k[:, :], 1.0)

            ones_m = cpool.tile([1, 128], F32)      # lhsT for K=1 broadcast MMs
            nc.vector.memset(ones_m[:, :], 1.0)
            iota_p = cpool.tile([128, 1], I32)      # value = partition index
            nc.gpsimd.iota(iota_p[:, :], pattern=[[0, 1]], base=0, channel_multiplier=1)
            zero_c = cpool.tile([128, 1], F32)
            nc.vector.memset(zero_c[:, :], 0.0)

            # int16 wrapped indices for dma_gather (pre-wrapped on host):
            # xw = raw vocab ids (gating), xws = compact slot ids (experts)
            xw = cpool.tile([128, BL, S // 16], I16)
            nc.sync.dma_start(out=xw[:, :, :], in_=xw_t[:, :, :])
            xws = cpool.tile([128, BL, S // 16], I16)
            nc.sync.dma_start(out=xws[:, :, :], in_=xws_t[:, :, :])

            gb1_sb = cpool.tile([128, MT], F32)
            nc.sync.dma_start(out=gb1_sb[:, :], in_=gb1_t[:, :])
            gb2_sb = cpool.tile([1, E], F32)
            nc.sync.dma_start(out=gb2_sb[:, :], in_=gb2_t[:, :])
            eyebl_sb = cpool.tile([GBL, GBL * 128], F32)
            nc.sync.dma_start(out=eyebl_sb[:, :], in_=eyebl_t[:, :])
            gw2_sb = cpool.tile([128, MT, E], F32)
            nc.sync.dma_start(
                out=gw2_sb[:, :, :], in_=gw2_t[:, :].rearrange("(m p) e -> p m e", p=128)
            )

            consts = dict(
                id_f=id_f, ones_k=ones_k,
                ones_m=ones_m, iota_p=iota_p, zero_c=zero_c, xw=xw, xws=xws,
                gb1_sb=gb1_sb, gb2_sb=gb2_sb, gw2_sb=gw2_sb, eyebl_sb=eyebl_sb,
            )
            tensors = dict(
                embg_t=embg_t, ctab_t=ctab_t, wx_t=wx_t, out_t=out_t,
            )
            # chain tile serializes reps so the benchmark differential is honest
            chain = None
            if reps > 1:
                chain = cpool.tile([1, 1], F32)
                nc.vector.memset(chain[:, :], 0.0)
            for rep in range(reps):
                _body_once(nc, tc, act, rep, dpool, consts, tensors, chain)

    nc.compile()
    return nc


def _body_once(nc, tc, act, rep, dpool, cn, tn, chain=None):
    sfx = f"_r{rep}"
    id_f = cn["id_f"]
    ones_k, ones_m, iota_p, zero_c = cn["ones_k"], cn["ones_m"], cn["iota_p"], cn["zero_c"]
    xw, xws = cn["xw"], cn["xws"]
    gb1_sb, gb2_sb, gw2_sb = cn["gb1_sb"], cn["gb2_sb"], cn["gw2_sb"]
    eyebl_sb = cn["eyebl_sb"]
    embg_t, ctab_t, wx_t, out_t = tn["embg_t"], tn["ctab_t"], tn["wx_t"], tn["out_t"]

    with (
        tc.tile_pool(name=f"persist{sfx}", bufs=1) as ppool,
        tc.tile_pool(name=f"bc{sfx}", bufs=2) as bcpool,
        # gating pools
        tc.tile_pool(name=f"gat{sfx}", bufs=4) as gpool,
        tc.tile_pool(name=f"gsb{sfx}", bufs=2) as gspool,
        tc.tile_pool(name=f"gps{sfx}", bufs=1, space="PSUM") as gps,
        tc.tile_pool(name=f"gpss{sfx}", bufs=2, space="PSUM") as gps_s,
        # expert pools
        tc.tile_pool(name=f"exi{sfx}", bufs=3) as xipool,
        tc.tile_pool(name=f"etok{sfx}", bufs=3) as tokpool,
        tc.tile_pool(name=f"ew{sfx}", bufs=3) as wpool,
        tc.tile_pool(name=f"esm{sfx}", bufs=3) as smpool,
        tc.tile_pool(name=f"ejunk{sfx}", bufs=2) as junkpool,
        tc.tile_pool(name=f"epsz{sfx}", bufs=4, space="PSUM") as eps_z,
        tc.tile_pool(name=f"epso{sfx}", bufs=1, space="PSUM") as eps_o,
    ):
        out_acc = ppool.tile([C, BL], F32)
        nc.vector.memset(out_acc[:, :], 0.0)

        # deferred W2 tail of the previous (b,k): emitting it after the next
        # (b,k)'s GEMMs keeps the in-order PE queue from stalling on the
        # relu->psc dependency
        pending = []

        def flush_tail():
            if not pending:
                return
            st = pending.pop()
            psc = smpool.tile([128, HT], F32, tag="psc")
            nc.vector.scalar_tensor_tensor(
                out=psc[:, :],
                in0=st["pacc"][:, :],
                scalar=FP8_UNSCALE / S,
                in1=st["BCf"][:, st["cRW"] : st["cRW"] + 1].to_broadcast([128, HT]),
                op0=mybir.AluOpType.mult,
                op1=mybir.AluOpType.mult,
            )
            eo_ps = eps_o.tile([C, 1], F32, tag="eo")
            for j2 in range(HT):
                nc.tensor.matmul(
                    out=eo_ps[:, :],
                    lhsT=st["w2f"][:, j2 * C : (j2 + 1) * C],
                    rhs=psc[:, j2 : j2 + 1],
                    start=(j2 == 0),
                    stop=(j2 == HT - 1),
                )
            # out_acc[:, b] += rw*(p@W2) + rw*b2: psc already carries rw, so
            # add rw*b2 via stt: (b2 mult rw) add eo
            eo2 = smpool.tile([C, 1], F32, tag="eo2")
            nc.vector.scalar_tensor_tensor(
                out=eo2[:, :],
                in0=st["b2f"][:, :],
                scalar=st["BCf"][0:C, st["cRW"] : st["cRW"] + 1],
                in1=eo_ps[:, :],
                op0=mybir.AluOpType.mult,
                op1=mybir.AluOpType.add,
            )
            b = st["b"]
            nc.vector.tensor_add(
                out_acc[:, b : b + 1], out_acc[:, b : b + 1], eo2[:, :]
            )

        for g in range(NGRP):
            b0 = g * GBL
            # ============ gating for samples [b0, b0+GBL) (f32) ============
            # hT[p, m] = relu(pooled @ gw1 + gb1)[m*128+p], computed directly
            # in partition layout: 8 tiny N=1 matmuls sum gtok g-slices over
            # tokens (f32r; m13 truncation is ~1e-8 on the logits, margins
            # are ~1e-5).
            hTs = gspool.tile([128, MT, GBL], F32, tag="hTs")
            for bl in range(GBL):
                b = b0 + bl
                # embG rows for this sample's tokens: [128, ST, 256]
                gtok = gpool.tile([128, ST, GATE_H], F32, tag="gtok")
                nc.gpsimd.dma_gather(
                    out_ap=gtok[:, :, :],
                    in_ap=embg_t[:, :],
                    idxs_ap=xw[:, b, :],
                    num_idxs=S,
                    num_idxs_reg=S,
                    elem_size=GATE_H,
                    transpose=False,
                )
                hp = gps.tile([128, MT], F32, tag="pp")
                for m in range(MT):
                    for t in range(ST):
                        nc.tensor.matmul(
                            out=hp[:, m : m + 1],
                            lhsT=gtok[:, t, m * 128 : (m + 1) * 128],
                            rhs=ones_k[:, :],
                            start=(t == 0),
                            stop=(t == ST - 1),
                        )
                # h = relu(hp + gb1)  (1/S is folded into embG on host)
                aT = gspool.tile([128, MT], F32, tag="aT")
                nc.vector.tensor_add(aT[:, :], hp[:, :], gb1_sb[:, :])
                nc.vector.tensor_scalar_max(hTs[:, :, bl], aT[:, :], 0.0)

            # gate layer 2 + gb2 (rank-1 matmul) -> logits [e, b], then
            # transpose to [b, e]; all on PE so the chain stays short
            l_ps = gps_s.tile([E, GBL], F32, tag="gmisc")
            for m in range(MT):
                nc.tensor.matmul(
                    out=l_ps[:, :],
                    lhsT=gw2_sb[:, m, :],
                    rhs=hTs[:, m, :],
                    start=(m == 0),
                    stop=False,
                )
            nc.tensor.matmul(
                out=l_ps[:, :],
                lhsT=gb2_sb[0:1, :],
                rhs=ones_m[0:1, 0:GBL],
                start=False,
                stop=True,
            )
            l_sb = gspool.tile([E, GBL], F32, tag="l_sb")
            nc.vector.tensor_copy(l_sb[:, :], l_ps[:, :])
            lt_ps = gps_s.tile([GBL, E], F32, tag="gmisc")
            nc.tensor.matmul(
                out=lt_ps[:, :], lhsT=l_sb[:, :], rhs=id_f[0:E, 0:E],
                start=True, stop=True,
            )

            # top-2 of logits == top-2 of softmax (monotone); DVE reads the
            # psum tile directly
            mx = gspool.tile([GBL, 8], F32, tag="mx")
            mi = gspool.tile([GBL, 8], U32, tag="mi")
            nc.vector.max_with_indices(mx[:, :], mi[:, :], lt_ps[:, :])

            # renormalized top-2 softmax weights:
            # rw1 = 1/(1+exp(l2-l1)), rw2 = exp(l2-l1)/(1+exp(l2-l1))
            dlt = gspool.tile([GBL, 1], F32, tag="dlt")
            nc.vector.tensor_sub(dlt[:, :], mx[:, 1:2], mx[:, 0:1])
            q = gspool.tile([GBL, 1], F32, tag="q")
            nc.scalar.activation(out=q[:, :], in_=dlt[:, :], func=act.Exp)
            sden = gspool.tile([GBL, 1], F32, tag="sden")
            nc.vector.tensor_scalar_add(sden[:, :], q[:, :], 1.0)
            rw1 = gspool.tile([GBL, 1], F32, tag="rw1")
            nc.vector.reciprocal(rw1[:, :], sden[:, :])
            rw2 = gspool.tile([GBL, 1], F32, tag="rw2")
            nc.vector.tensor_mul(rw2[:, :], q[:, :], rw1[:, :])

            # pack per-(b,k) scalars: cols bl*8 + {0,1}=e*U, {2,3}=e*128,
            # {6,7}=rw ({4,5} unused)
            ei_f = gspool.tile([GBL, TOPK], F32, tag="ei_f")
            nc.vector.tensor_copy(ei_f[:, :], mi[:, 0:TOPK])
            vals = gspool.tile([GBL, 8], F32, tag="vals")
            nc.vector.tensor_scalar_mul(vals[:, 0:2], ei_f[:, :], float(U))
            nc.vector.tensor_scalar_mul(vals[:, 2:4], ei_f[:, :], 128.0)
            nc.vector.tensor_scalar_mul(vals[:, 4:6], ei_f[:, :], 0.0)
            nc.vector.tensor_copy(vals[:, 6:7], rw1[:, :])
            nc.vector.tensor_copy(vals[:, 7:8], rw2[:, :])

            # broadcast vals[bl, :] to all partitions of cols bl*8..bl*8+8
            # via one-hot-row lhsT matmuls (no DRAM bounce)
            if chain is not None:
                # unused col 4: forces rep r to wait on rep r-1's result
                nc.vector.tensor_copy(vals[0:1, 4:5], chain[0:1, 0:1])
            bc_ps = gps_s.tile([128, GBL * 8], F32, tag="gmisc")
            for bl in range(GBL):
                nc.tensor.matmul(
                    out=bc_ps[:, bl * 8 : (bl + 1) * 8],
                    lhsT=eyebl_sb[:, bl * 128 : (bl + 1) * 128],
                    rhs=vals[:, :],
                    start=True,
                    stop=True,
                )
            BCf = bcpool.tile([128, GBL * 8], F32, tag="bcf")
            BCi = bcpool.tile([128, GBL * 8], I32, tag="bci")
            BCi16 = bcpool.tile([128, GBL * 8], I16, tag="bci16")
            nc.vector.tensor_copy(BCf[:, :], bc_ps[:, :])
            nc.vector.tensor_copy(BCi[:, :], bc_ps[:, :])    # cast f32->i32
            nc.vector.tensor_copy(BCi16[:, :], bc_ps[:, :])  # cast f32->i16

            # ============ experts for this group (fp8) ============
            for bl in range(GBL):
                b = b0 + bl
                for k in range(TOPK):
                    cEV = bl * 8 + k
                    cE128 = bl * 8 + 2 + k
                    cRW = bl * 8 + 6 + k

                    # compact-table indices: slot + e*U (fits int16: <= 32763)
                    tok_idx = xipool.tile([128, S // 16], I16, tag="tok_idx")
                    nc.vector.tensor_add(
                        tok_idx[:, :],
                        xws[:, b, :],
                        BCi16[:, cEV : cEV + 1].to_broadcast([128, S // 16]),
                    )
                    w_idx = xipool.tile([128, 1], I32, tag="w_idx")
                    nc.vector.tensor_add(
                        w_idx[:, :], iota_p[:, :], BCi[:, cE128 : cE128 + 1]
                    )

                    # transposed gather (u16 granularity is exact in the i16
                    # view): tok16[p, jj, i] = row16_i[jj*128 + p]. The host
                    # byte-permutes each table row so u16 unit (jj*128+p)
                    # holds the fp8 pair (d, d+512) with d = jj*128+p — the
                    # DoubleRow rhs then reads [p, l(stride 1), i(stride 2)].
                    tok16 = tokpool.tile([128, ST, S], I16, tag="tok")
                    nc.gpsimd.dma_gather(
                        out_ap=tok16[:, :, :],
                        in_ap=ctab_t[:, :].bitcast(I16),
                        idxs_ap=tok_idx[:, :],
                        num_idxs=S,
                        num_idxs_reg=S,
                        elem_size=D // 2,
                        transpose=True,
                    )
                    # one gather for W1 (fp8) + bf16 side table (as raw bytes)
                    wg = wpool.tile([128, WXCOLS], F8, tag="wg")
                    nc.gpsimd.indirect_dma_start(
                        out=wg[:, :],
                        out_offset=None,
                        in_=wx_t[:, :],
                        in_offset=IndirectOffsetOnAxis(ap=w_idx[:, :], axis=0),
                    )
                    wsm = wg[:, W1OFF:].bitcast(BF16)      # [128, WSMCOLS] bf16
                    b1un = smpool.tile([128, 2 * HT], F32, tag="b1un")
                    nc.vector.tensor_copy(b1un[:, :], wsm[:, B1COL : B1COL + 2 * HT])
                    b1u = b1un[:, 0:HT]
                    b1n = b1un[:, HT : 2 * HT]
                    b2f = smpool.tile([C, 1], F32, tag="b2f")
                    nc.vector.tensor_copy(b2f[:, :], wsm[0:C, B2COL : B2COL + 1])
                    w2f = smpool.tile([128, HT * C], F32, tag="w2f")
                    w2f_eng = nc.gpsimd if W2F_ENG == "P" else nc.vector
                    w2f_eng.tensor_add(
                        w2f[:, :], wsm[:, W2COL : W2COL + HT * C],
                        wsm[:, W2LO : W2LO + HT * C],
                    )

                    # z[h_tile] = relu(tokT.T @ W1 + b1*SC^2); accumulate sum
                    # over s. fp8 DoubleRow contracts the (d, d+512) pair of
                    # each u16 unit per matmul; W1 is host-packed to match.
                    w1v = wg[:, 0:W1OFF].rearrange("p (kk h) -> p kk h", kk=DT)
                    pacc = smpool.tile([128, HT], F32, tag="pacc")
                    for j2 in range(HT):
                        z_ps = eps_z.tile([128, S], F32, tag="z")
                        for jj in range(ST):
                            rhs = (
                                tok16[:, jj, :].bitcast(F8)
                                .rearrange("p (i l) -> p l i", l=2)
                            )
                            nc.tensor.matmul(
                                out=z_ps[:, :],
                                lhsT=w1v[:, 2 * jj : 2 * jj + 2,
                                         j2 * 128 : (j2 + 1) * 128],
                                rhs=rhs,
                                start=(jj == 0),
                                stop=(jj == ST - 1),
                                perf_mode=mybir.MatmulPerfMode.DoubleRow,
                            )
                        zjunk = junkpool.tile([128, S], BF16, tag="zjunk")
                        if RELU_ENG[j2] == "A":
                            # scalar engine: relu(z + b1u), accum over s
                            nc.scalar.activation(
                                out=zjunk[:, :],
                                in_=z_ps[:, :],
                                func=act.Relu,
                                bias=b1u[:, j2 : j2 + 1],
                                accum_out=pacc[:, j2 : j2 + 1],
                            )
                        else:
                            # DVE: relu(z + c) = max(z, -c) + c, accum over s
                            nc.vector.scalar_tensor_tensor(
                                out=zjunk[:, :],
                                in0=z_ps[:, :],
                                scalar=b1n[:, j2 : j2 + 1],
                                in1=b1u[:, j2 : j2 + 1].to_broadcast([128, S]),
                                op0=mybir.AluOpType.max,
                                op1=mybir.AluOpType.add,
                                accum_out=pacc[:, j2 : j2 + 1],
                            )

                    pending.append(
                        dict(pacc=pacc, w2f=w2f, b2f=b2f, BCf=BCf, cRW=cRW, b=b)
                    )
                    flush_tail()

        flush_tail()
        if chain is not None:
            nc.vector.tensor_copy(chain[0:1, 0:1], out_acc[0:1, 0:1])
        nc.sync.dma_start(
            out=out_t[:, :].rearrange("b c -> c b"), in_=out_acc[:, :]
        )


def _prep_inputs(inputs):
    """Host-side dtype casts + re-layouts shared by all cores."""
    import ml_dtypes

    f32 = np.float32
    bf16 = ml_dtypes.bfloat16
    fp8 = ml_dtypes.float8_e4m3

    def wrap16(ids):
        """[BL, S] int -> [128, BL, S/16] int16 wrapped for dma_gather."""
        w = ids.reshape(BL, S // 16, 16).transpose(2, 0, 1).astype(np.int16)
        return np.ascontiguousarray(np.tile(w, (8, 1, 1)))

    x = np.asarray(inputs["x"]).astype(np.int32)

    # gating: pre-multiply emb through gate_w1 (and fold 1/S)
    emb = np.asarray(inputs["emb"], dtype=np.float64)
    gw1 = np.asarray(inputs["gate_w1"], dtype=np.float64)
    embg = np.ascontiguousarray((emb @ gw1) / S).astype(f32)            # [V, 256]

    exp_emb = np.clip(
        np.asarray(inputs["exp_emb"], dtype=f32) * FP8_SCALE, -240.0, 240.0
    ).astype(fp8)                                                       # [E, V, D]
    # byte-permute each row so the u16-granularity transposed gather lands
    # d-major: stored u16 unit (jj*128+p) = fp8 pair (d, d+512), d=jj*128+p
    beta = np.arange(D)
    u, lo = beta // 2, beta % 2
    dperm = lo * 512 + (u // 128) * 128 + (u % 128)
    exp_emb = np.ascontiguousarray(exp_emb[:, :, dperm])

    # per-core compact expert-embedding table: each core touches at most
    # BL*S = U distinct vocab ids, so (e, slot) indices fit in int16
    percore = []
    for c in range(NCORES):
        xc = x[c * BL : (c + 1) * BL]                                   # [BL, S]
        uniq, inv = np.unique(xc, return_inverse=True)
        upad = np.zeros(U, np.int64)
        upad[: uniq.size] = uniq
        ctab = np.ascontiguousarray(
            exp_emb[:, upad, :].reshape(E * U, D)
        )
        percore.append(
            dict(
                xw16=wrap16(xc),
                xws16=wrap16(inv.reshape(BL, S)),
                ctab=ctab,
            )
        )

    # W1 packed to match the pair layout: col (jj*2 + l)*H + h on partition p
    # holds W1[l*512 + jj*128 + p, h]
    w1 = np.asarray(inputs["exp_w1"], dtype=f32)          # [E, D, H]
    ew1 = (
        w1.reshape(E, 2, ST, 128, H)                      # [e, l, jj, p, h]
        .transpose(0, 3, 2, 1, 4)                         # [e, p, jj, l, h]
        .reshape(E * 128, DT * H)
    )
    w1all = np.ascontiguousarray(
        np.clip(ew1 * FP8_SCALE, -240.0, 240.0)
    ).astype(fp8)
    w2 = np.asarray(inputs["exp_w2"], dtype=f32)          # [E, H, C]
    ew2 = w2.reshape(E, HT, 128, C).transpose(0, 2, 1, 3).reshape(E * 128, HT * C)
    b1 = np.asarray(inputs["exp_b1"], dtype=f32)          # [E, H]
    b1r = b1.reshape(E, HT, 128).transpose(0, 2, 1).reshape(E * 128, HT)
    b2 = np.asarray(inputs["exp_b2"], dtype=f32)          # [E, C]
    b2slot = np.zeros((E * 128, 1), f32)
    for e in range(E):
        b2slot[e * 128 : e * 128 + C, 0] = b2[e]
    w2hi = ew2.astype(bf16).astype(f32)
    w2lo = ew2 - w2hi
    wsm = np.zeros((E * 128, WSMCOLS), f32)
    wsm[:, W2COL : W2COL + HT * C] = w2hi
    wsm[:, W2LO : W2LO + HT * C] = w2lo
    wsm[:, B1COL : B1COL + HT] = b1r * (FP8_SCALE * FP8_SCALE)
    wsm[:, B2COL : B2COL + 1] = b2slot
    wsm[:, B1NEG : B1NEG + HT] = -b1r * (FP8_SCALE * FP8_SCALE)
    wsm8 = np.ascontiguousarray(wsm).astype(bf16).view(fp8)             # [E*128, 544]
    wx = np.ascontiguousarray(np.concatenate([w1all, wsm8], axis=1))    # [E*128, 8736]

    gb1 = np.ascontiguousarray(
        np.asarray(inputs["gate_b1"], dtype=f32).reshape(MT, 128).T
    )
    gw2 = np.ascontiguousarray(np.asarray(inputs["gate_w2"], dtype=f32))
    gb2r = np.ascontiguousarray(np.asarray(inputs["gate_b2"], dtype=f32).reshape(1, E))
    eyebl = np.zeros((GBL, GBL * 128), f32)
    for bl in range(GBL):
        eyebl[bl, bl * 128 : (bl + 1) * 128] = 1.0

    shared = dict(
        embg=embg, wx=wx,
        gb1=gb1, gw2=gw2, gb2r=gb2r, eyebl=eyebl,
    )
    return percore, shared


def kernel(**inputs) -> np.ndarray:
    global last_results
    if "nc" not in _compiled:
        _compiled["nc"] = build_program()
    nc = _compiled["nc"]

    percore, shared = _prep_inputs(inputs)
    in_maps = [{**percore[c], **shared} for c in range(NCORES)]
    trace = os.environ.get("KERNEL_TRACE", "0") == "1"
    kw = {}
    if trace:
        tdir = os.environ.get("KERNEL_TRACE_DIR", "/root/problem/trace_out")
        os.makedirs(tdir, exist_ok=True)
        kw = dict(trace=True, tmpdir=tdir)
    res = run_bass_kernel_spmd(nc, in_maps, list(range(NCORES)), **kw)
    last_results = res
    out = np.concatenate([res.results[c]["out"] for c in range(NCORES)], axis=0)
    return np.ascontiguousarray(out.astype(np.float32))
